# revision 1
# baseline (speedup 1.0000x reference)
"""Local 7x7-window per-channel attention (SASA-style) on 8 TRN2 NeuronCores.

Reference computation per (batch, channel, pixel):
  q = groupconv1x1(x, wq);  k = groupconv1x1(pad(x), wk) + bk;  v = likewise wv/bv
  logits[k_off] = q * (k[p + k_off] + r_c[k_off])     (49 window offsets)
  out = sum_k softmax(logits)[k] * v[p + k_off]
where r_c[kh,kw] = rel_x[d,kh] for channel-in-group d<4, rel_y[d-4,kw] for d>=4.

Sharding: pure data-parallel. Core c owns image b=c//2, output-row half
h=c%2 (28 rows). On-core, the half is split into two 14-row quarters
stacked on SBUF partitions: partition p = quarter*64 + channel.
Each quarter's padded input slab is (64ch, 20rows, 62cols); no collectives.
"""

import sys

if "/opt/trn_rl_repo" not in sys.path:
    sys.path.insert(0, "/opt/trn_rl_repo")

import numpy as np

import concourse.bass as bass
import concourse.bacc as bacc
import concourse.tile as tile
from concourse import mybir
from concourse.bass_utils import run_bass_kernel_spmd

N_CORES = 8
KS = 7
PAD = 3
G = 8
DD = 8
C = 64  # channels
H = W = 56
B = 4
QR = 14          # output rows per quarter
PR = QR + 2 * PAD  # padded rows per quarter slab = 20
PW = W + 2 * PAD   # padded width = 62
CH = 7           # chunk rows for the attention loop
NCHUNK = QR // CH

F32 = mybir.dt.float32
ALU = mybir.AluOpType
ACTF = mybir.ActivationFunctionType


def build_nc():
    nc = bacc.Bacc("TRN2", target_bir_lowering=False, debug=False,
                   num_devices=N_CORES)
    x_ap = nc.dram_tensor("x", [128, PR, PW], F32, kind="ExternalInput").ap()
    wq_ap = nc.dram_tensor("wq", [128, 128], F32, kind="ExternalInput").ap()
    wk_ap = nc.dram_tensor("wk", [128, 128], F32, kind="ExternalInput").ap()
    wv_ap = nc.dram_tensor("wv", [128, 128], F32, kind="ExternalInput").ap()
    bk_ap = nc.dram_tensor("bk", [128, 1], F32, kind="ExternalInput").ap()
    bv_ap = nc.dram_tensor("bv", [128, 1], F32, kind="ExternalInput").ap()
    rt_ap = nc.dram_tensor("rt", [128, KS * KS], F32, kind="ExternalInput").ap()
    out_ap = nc.dram_tensor("out", [128, QR, W], F32, kind="ExternalOutput").ap()

    with tile.TileContext(nc) as tc:
        with tc.tile_pool(name="const", bufs=1) as constp, \
             tc.tile_pool(name="planes", bufs=1) as planesp, \
             tc.tile_pool(name="big", bufs=2) as bigp, \
             tc.tile_pool(name="small", bufs=2) as smallp, \
             tc.tile_pool(name="psum", bufs=2, space="PSUM") as psump:

            X = planesp.tile([128, PR, PW], F32)
            nc.sync.dma_start(out=X[:], in_=x_ap[:])
            Wq = constp.tile([128, 128], F32)
            nc.sync.dma_start(out=Wq[:], in_=wq_ap[:])
            Wk = constp.tile([128, 128], F32)
            nc.sync.dma_start(out=Wk[:], in_=wk_ap[:])
            Wv = constp.tile([128, 128], F32)
            nc.sync.dma_start(out=Wv[:], in_=wv_ap[:])
            BK = constp.tile([128, 1], F32)
            nc.sync.dma_start(out=BK[:], in_=bk_ap[:])
            BV = constp.tile([128, 1], F32)
            nc.sync.dma_start(out=BV[:], in_=bv_ap[:])
            RT = constp.tile([128, KS * KS], F32)
            nc.sync.dma_start(out=RT[:], in_=rt_ap[:])

            K = planesp.tile([128, PR, PW], F32)
            V = planesp.tile([128, PR, PW], F32)
            Q = planesp.tile([128, QR, W], F32)

            # k / v projections over the whole padded slab (1240 cols, 4x310)
            Xflat = X[:].rearrange("p h w -> p (h w)")
            Kflat = K[:].rearrange("p h w -> p (h w)")
            Vflat = V[:].rearrange("p h w -> p (h w)")
            ncol = PR * PW
            step = 310
            for (dst, wmat, bias) in ((Kflat, Wk, BK), (Vflat, Wv, BV)):
                for j in range(0, ncol, step):
                    ps = psump.tile([128, step], F32, tag="ps")
                    nc.tensor.matmul(ps[:], wmat[:], Xflat[:, j:j + step],
                                     start=True, stop=True)
                    nc.scalar.add(out=dst[:, j:j + step], in_=ps[:], add=bias[:])
            # q projection on the interior only (14x56, 2x392)
            for j in range(2):
                ps = psump.tile([128, CH * W], F32, tag="ps")
                nc.tensor.matmul(
                    ps[:], Wq[:],
                    X[:, PAD + j * CH: PAD + (j + 1) * CH, PAD:PAD + W],
                    start=True, stop=True)
                nc.scalar.copy(
                    out=Q[:, j * CH:(j + 1) * CH, :].rearrange("p h w -> p (h w)"),
                    in_=ps[:])

            # attention over row chunks
            for chunk in range(NCHUNK):
                r0 = chunk * CH
                P = CH * W
                L = bigp.tile([128, KS * KS, CH, W], F32, tag="L")
                for kh in range(KS):
                    for kw in range(KS):
                        k = kh * KS + kw
                        nc.vector.scalar_tensor_tensor(
                            out=L[:, k],
                            in0=K[:, r0 + kh:r0 + kh + CH, kw:kw + W],
                            scalar=RT[:, k:k + 1],
                            in1=Q[:, r0:r0 + CH, :],
                            op0=ALU.add,
                            op1=ALU.mult)
                Lflat = L[:].rearrange("p k h w -> p (k h w)")
                nc.scalar.activation(out=Lflat, in_=Lflat, func=ACTF.Exp)
                S = smallp.tile([128, P], F32, tag="S")
                nc.vector.tensor_reduce(
                    out=S[:], in_=L[:].rearrange("p k h w -> p (h w) k"),
                    axis=mybir.AxisListType.X, op=ALU.add)
                for kh in range(KS):
                    for kw in range(KS):
                        k = kh * KS + kw
                        nc.vector.tensor_mul(
                            L[:, k], L[:, k],
                            V[:, r0 + kh:r0 + kh + CH, kw:kw + W])
                O = smallp.tile([128, P], F32, tag="O")
                nc.vector.tensor_reduce(
                    out=O[:], in_=L[:].rearrange("p k h w -> p (h w) k"),
                    axis=mybir.AxisListType.X, op=ALU.add)
                R = smallp.tile([128, P], F32, tag="R")
                nc.vector.reciprocal(out=R[:], in_=S[:])
                OUTC = smallp.tile([128, P], F32, tag="OUTC")
                nc.vector.tensor_mul(OUTC[:], O[:], R[:])
                nc.sync.dma_start(
                    out=out_ap[:, r0:r0 + CH, :],
                    in_=OUTC[:].rearrange("p (h w) -> p h w", h=CH))

    nc.compile()
    return nc


def shard_inputs(x, wq, wk, bk, wv, bv, rel_x, rel_y):
    """Full inputs -> list of 8 per-core input dicts (pure indexing/reshape)."""
    x_pad = np.zeros((B, C, H + 2 * PAD, W + 2 * PAD), dtype=np.float32)
    x_pad[:, :, PAD:PAD + H, PAD:PAD + W] = x

    def blockdiag(w):
        # lhsT layout: [cin, cout]; W64[g*8+i, g*8+o] = w[g, o, i]
        w64 = np.zeros((C, C), dtype=np.float32)
        for g in range(G):
            w64[g * DD:(g + 1) * DD, g * DD:(g + 1) * DD] = w[g].T
        w128 = np.zeros((128, 128), dtype=np.float32)
        w128[:64, :64] = w64
        w128[64:, 64:] = w64
        return w128

    wq128, wk128, wv128 = blockdiag(wq), blockdiag(wk), blockdiag(wv)
    bk128 = np.concatenate([bk, bk]).reshape(128, 1).astype(np.float32)
    bv128 = np.concatenate([bv, bv]).reshape(128, 1).astype(np.float32)

    rt64 = np.empty((C, KS, KS), dtype=np.float32)
    for g in range(G):
        for d in range(DD):
            if d < DD // 2:
                rt64[g * DD + d] = rel_x[d]          # (7,1) -> broadcast cols
            else:
                rt64[g * DD + d] = rel_y[d - DD // 2]  # (1,7) -> broadcast rows
    rt128 = np.concatenate([rt64, rt64]).reshape(128, KS * KS)
    rt128 = np.ascontiguousarray(rt128, dtype=np.float32)

    in_maps = []
    for core in range(N_CORES):
        b, half = divmod(core, 2)
        r0 = half * 2 * QR
        xs = np.empty((128, PR, PW), dtype=np.float32)
        xs[:64] = x_pad[b, :, r0:r0 + PR, :]
        xs[64:] = x_pad[b, :, r0 + QR:r0 + QR + PR, :]
        in_maps.append({
            "x": xs, "wq": wq128, "wk": wk128, "wv": wv128,
            "bk": bk128, "bv": bv128, "rt": rt128,
        })
    return in_maps


def unshard_output(results):
    out = np.empty((B, C, H, W), dtype=np.float32)
    for core in range(N_CORES):
        b, half = divmod(core, 2)
        r0 = half * 2 * QR
        r = results[core]["out"]  # (128, 14, 56)
        out[b, :, r0:r0 + QR, :] = r[:64]
        out[b, :, r0 + QR:r0 + 2 * QR, :] = r[64:]
    return out


_NC_CACHE = {}


def get_nc():
    if "nc" not in _NC_CACHE:
        _NC_CACHE["nc"] = build_nc()
    return _NC_CACHE["nc"]


def kernel(**inputs):
    nc = get_nc()
    in_maps = shard_inputs(**inputs)
    res = run_bass_kernel_spmd(nc, in_maps, core_ids=list(range(N_CORES)))
    return unshard_output(res.results)


# revision 8
# speedup vs baseline: 2.2158x; 2.2158x over previous
"""Local 7x7-window per-channel attention (SASA-style) on 8 TRN2 NeuronCores.

Reference computation per (batch, channel, pixel):
  q = groupconv1x1(x, wq);  k = groupconv1x1(pad(x), wk) + bk;  v = likewise wv/bv
  logits[k_off] = q * (k[p + k_off] + r_c[k_off])     (49 window offsets)
  out = sum_k softmax(logits)[k] * v[p + k_off]
where r_c[kh,kw] = rel_x[d,kh] for channel-in-group d<4, rel_y[d-4,kw] for d>=4.

Sharding: pure data-parallel. Core c owns image b=c//2, output-row half
h=c%2 (28 rows). On-core, the half is split into two 14-row quarters
stacked on SBUF partitions: partition p = quarter*64 + channel.
Each quarter's padded input slab is (64ch, 20rows, 62cols); no collectives.
"""

import sys

if "/opt/trn_rl_repo" not in sys.path:
    sys.path.insert(0, "/opt/trn_rl_repo")

import numpy as np

import concourse.bass as bass
import concourse.bacc as bacc
import concourse.tile as tile
from concourse import mybir
from concourse.bass_utils import run_bass_kernel_spmd

N_CORES = 8
KS = 7
PAD = 3
G = 8
DD = 8
C = 64  # channels
H = W = 56
B = 4
QR = 14          # output rows per quarter
PR = QR + 2 * PAD  # padded rows per quarter slab = 20
PW = W + 2 * PAD   # padded width = 62
CH = 7           # chunk rows for the attention loop
NCHUNK = QR // CH

F32 = mybir.dt.float32
BF16 = mybir.dt.bfloat16
ALU = mybir.AluOpType
ACTF = mybir.ActivationFunctionType


def _tree_fold(nc, T, nplanes):
    """Sum planes T[:, 0:nplanes] into T[:, 0] with in-place pairwise adds.

    T is a tile AP of shape (128, nplanes, R, C). Fat adds keep DVE in its
    2x bf16 mode; fp32 happens inside the ALU, rounding only at each store.
    """
    live = nplanes
    while live > 1:
        half = live // 2
        rem = live - 2 * half  # 0 or 1
        nc.vector.tensor_tensor(
            T[:, 0:half], T[:, 0:half], T[:, half:2 * half], ALU.add)
        if rem:
            if half >= 1:
                # fold the odd plane into plane 0 range next round
                nc.vector.tensor_tensor(
                    T[:, 0:1], T[:, 0:1], T[:, 2 * half:2 * half + 1], ALU.add)
        live = half


def build_nc():
    nc = bacc.Bacc("TRN2", target_bir_lowering=False, debug=False,
                   num_devices=N_CORES)
    x_ap = nc.dram_tensor("x", [128, PR, PW], F32, kind="ExternalInput").ap()
    wq_ap = nc.dram_tensor("wq", [128, 128], F32, kind="ExternalInput").ap()
    wk_ap = nc.dram_tensor("wk", [128, 128], F32, kind="ExternalInput").ap()
    wv_ap = nc.dram_tensor("wv", [128, 128], F32, kind="ExternalInput").ap()
    bk_ap = nc.dram_tensor("bk", [128, 1], F32, kind="ExternalInput").ap()
    bv_ap = nc.dram_tensor("bv", [128, 1], F32, kind="ExternalInput").ap()
    rt_ap = nc.dram_tensor("rt", [128, KS * KS], F32, kind="ExternalInput").ap()
    out_ap = nc.dram_tensor("out", [128, QR, W], F32, kind="ExternalOutput").ap()

    with tile.TileContext(nc) as tc:
        with tc.tile_pool(name="const", bufs=1) as constp, \
             tc.tile_pool(name="planes", bufs=1) as planesp, \
             tc.tile_pool(name="big", bufs=1) as bigp, \
             tc.tile_pool(name="small", bufs=2) as smallp, \
             tc.tile_pool(name="psum", bufs=2, space="PSUM") as psump:

            X = planesp.tile([128, PR, PW], F32)
            nc.sync.dma_start(out=X[:], in_=x_ap[:])
            Wq = constp.tile([128, 128], F32)
            nc.sync.dma_start(out=Wq[:], in_=wq_ap[:])
            Wk = constp.tile([128, 128], F32)
            nc.sync.dma_start(out=Wk[:], in_=wk_ap[:])
            Wv = constp.tile([128, 128], F32)
            nc.sync.dma_start(out=Wv[:], in_=wv_ap[:])
            BK = constp.tile([128, 1], F32)
            nc.sync.dma_start(out=BK[:], in_=bk_ap[:])
            BV = constp.tile([128, 1], F32)
            nc.sync.dma_start(out=BV[:], in_=bv_ap[:])
            RT = constp.tile([128, KS * KS], F32)
            nc.sync.dma_start(out=RT[:], in_=rt_ap[:])

            K = planesp.tile([128, PR, PW], F32)
            V = planesp.tile([128, PR, PW], BF16)
            Q = planesp.tile([128, QR, W], BF16)

            # k / v projections over the whole padded slab (1240 cols, 4x310)
            Xflat = X[:].rearrange("p h w -> p (h w)")
            Kflat = K[:].rearrange("p h w -> p (h w)")
            Vflat = V[:].rearrange("p h w -> p (h w)")
            ncol = PR * PW
            step = 310
            for (dst, wmat, bias) in ((Kflat, Wk, BK), (Vflat, Wv, BV)):
                for j in range(0, ncol, step):
                    ps = psump.tile([128, step], F32, tag="ps")
                    nc.tensor.matmul(ps[:], wmat[:], Xflat[:, j:j + step],
                                     start=True, stop=True)
                    nc.scalar.add(out=dst[:, j:j + step], in_=ps[:], add=bias[:])
            # q projection on the interior only (14x56, 2x392)
            for j in range(2):
                ps = psump.tile([128, CH * W], F32, tag="ps")
                nc.tensor.matmul(
                    ps[:], Wq[:],
                    X[:, PAD + j * CH: PAD + (j + 1) * CH, PAD:PAD + W],
                    start=True, stop=True)
                nc.scalar.copy(
                    out=Q[:, j * CH:(j + 1) * CH, :].rearrange("p h w -> p (h w)"),
                    in_=ps[:])

            # attention: single 14-row chunk, bf16 logits/weights.
            # Per kh-block pipeline keeps ACT (r-add + exp) and DVE
            # (q-mult, v-mult, folds) overlapped across blocks.
            L = bigp.tile([128, KS * KS, QR, W], BF16, tag="L")
            EV = bigp.tile([128, KS * KS, QR, W], BF16, tag="EV")
            qap = Q[:]
            qbcast = bass.AP(
                tensor=qap.tensor, offset=qap.offset,
                ap=[qap.ap[0], [0, KS], [W, QR], [1, W]])
            vap = V[:]

            def block_fold(T, b0):
                # planes b0..b0+6 summed into plane b0 (in place)
                nc.vector.tensor_tensor(
                    T[:, b0:b0 + 3], T[:, b0:b0 + 3], T[:, b0 + 3:b0 + 6],
                    ALU.add)
                for j in (1, 2, 6):
                    nc.vector.tensor_tensor(
                        T[:, b0:b0 + 1], T[:, b0:b0 + 1],
                        T[:, b0 + j:b0 + j + 1], ALU.add)

            for kh in range(KS):
                b0 = kh * KS
                # r-add on ScalarE: L[k] = K_win + r_k
                for kw in range(KS):
                    k = b0 + kw
                    nc.scalar.activation(
                        out=L[:, k], in_=K[:, kh:kh + QR, kw:kw + W],
                        func=ACTF.Identity, bias=RT[:, k:k + 1])
                blk = L[:, b0:b0 + KS]
                nc.vector.tensor_tensor(blk, blk, qbcast, ALU.mult)
                eblk = blk.rearrange("p k h w -> p (k h w)")
                nc.scalar.activation(out=eblk, in_=eblk, func=ACTF.Exp)
                vwin = bass.AP(
                    tensor=vap.tensor, offset=vap.offset + kh * PW,
                    ap=[vap.ap[0], [1, KS], [PW, QR], [1, W]])
                nc.vector.tensor_tensor(EV[:, b0:b0 + KS], blk, vwin, ALU.mult)
                block_fold(EV, b0)
                block_fold(L, b0)

            # cross-block fold over planes {0, 7, ..., 42} (k-stride 7)
            def stride_planes(T, start, n):
                t = T[:]
                return bass.AP(
                    tensor=t.tensor,
                    offset=t.offset + start * KS * QR * W,
                    ap=[t.ap[0], [KS * QR * W, n], [W, QR], [1, W]])

            for T in (EV, L):
                nc.vector.tensor_tensor(
                    stride_planes(T, 0, 3), stride_planes(T, 0, 3),
                    stride_planes(T, 3, 3), ALU.add)
                for j in (1, 2, 6):
                    nc.vector.tensor_tensor(
                        stride_planes(T, 0, 1), stride_planes(T, 0, 1),
                        stride_planes(T, j, 1), ALU.add)
            P = QR * W
            Sf = L[:, 0].rearrange("p h w -> p (h w)")
            Of = EV[:, 0].rearrange("p h w -> p (h w)")
            # 1/S on ScalarE: exp(-ln S); Exp and Log share one table set
            LNS = smallp.tile([128, P], F32, tag="LNS")
            nc.scalar.activation(out=LNS[:], in_=Sf, func=ACTF.Ln)
            R = smallp.tile([128, P], F32, tag="R")
            nc.scalar.activation(out=R[:], in_=LNS[:], func=ACTF.Exp, scale=-1.0)
            OUTC = smallp.tile([128, P], F32, tag="OUTC")
            nc.vector.tensor_mul(OUTC[:], Of, R[:])
            nc.sync.dma_start(
                out=out_ap[:],
                in_=OUTC[:].rearrange("p (h w) -> p h w", h=QR))

    nc.compile()
    return nc


def shard_inputs(x, wq, wk, bk, wv, bv, rel_x, rel_y):
    """Full inputs -> list of 8 per-core input dicts (pure indexing/reshape)."""
    x_pad = np.zeros((B, C, H + 2 * PAD, W + 2 * PAD), dtype=np.float32)
    x_pad[:, :, PAD:PAD + H, PAD:PAD + W] = x

    def blockdiag(w):
        # lhsT layout: [cin, cout]; W64[g*8+i, g*8+o] = w[g, o, i]
        w64 = np.zeros((C, C), dtype=np.float32)
        for g in range(G):
            w64[g * DD:(g + 1) * DD, g * DD:(g + 1) * DD] = w[g].T
        w128 = np.zeros((128, 128), dtype=np.float32)
        w128[:64, :64] = w64
        w128[64:, 64:] = w64
        return w128

    wq128, wk128, wv128 = blockdiag(wq), blockdiag(wk), blockdiag(wv)
    bk128 = np.concatenate([bk, bk]).reshape(128, 1).astype(np.float32)
    bv128 = np.concatenate([bv, bv]).reshape(128, 1).astype(np.float32)

    rt64 = np.empty((C, KS, KS), dtype=np.float32)
    for g in range(G):
        for d in range(DD):
            if d < DD // 2:
                rt64[g * DD + d] = rel_x[d]          # (7,1) -> broadcast cols
            else:
                rt64[g * DD + d] = rel_y[d - DD // 2]  # (1,7) -> broadcast rows
    rt128 = np.concatenate([rt64, rt64]).reshape(128, KS * KS)
    rt128 = np.ascontiguousarray(rt128, dtype=np.float32)

    in_maps = []
    for core in range(N_CORES):
        b, half = divmod(core, 2)
        r0 = half * 2 * QR
        xs = np.empty((128, PR, PW), dtype=np.float32)
        xs[:64] = x_pad[b, :, r0:r0 + PR, :]
        xs[64:] = x_pad[b, :, r0 + QR:r0 + QR + PR, :]
        in_maps.append({
            "x": xs, "wq": wq128, "wk": wk128, "wv": wv128,
            "bk": bk128, "bv": bv128, "rt": rt128,
        })
    return in_maps


def unshard_output(results):
    out = np.empty((B, C, H, W), dtype=np.float32)
    for core in range(N_CORES):
        b, half = divmod(core, 2)
        r0 = half * 2 * QR
        r = results[core]["out"]  # (128, 14, 56)
        out[b, :, r0:r0 + QR, :] = r[:64]
        out[b, :, r0 + QR:r0 + 2 * QR, :] = r[64:]
    return out


_NC_CACHE = {}


def get_nc():
    if "nc" not in _NC_CACHE:
        _NC_CACHE["nc"] = build_nc()
    return _NC_CACHE["nc"]


def kernel(**inputs):
    nc = get_nc()
    in_maps = shard_inputs(**inputs)
    res = run_bass_kernel_spmd(nc, in_maps, core_ids=list(range(N_CORES)))
    return unshard_output(res.results)
